# revision 11
# baseline (speedup 1.0000x reference)
"""Multi-head causal attention (B=2, T=2048, D=1024, H=16) on 8 TRN2
NeuronCores: data parallel over batch x tensor parallel over head groups
(4 heads per core). Each core computes its group's Q/K/V projections,
causal attention, and a partial output projection in bf16; the host sums
the 4 bf16 partials per batch element in fp32.

Key design points:
- inputs pre-cast to bf16 and pre-arranged partition-major on the host so
  every DMA runs 4-8KB contiguous descriptors across both HW DGE queues
- PE warmup matmuls un-throttle the HAM clock gate during the input DMA
- S-matmuls packed 2 heads per PE pass via tile_position row groups
  (K=64 each, concurrent in rows 0:64 / 64:128)
- V^T computed directly (x as the stationary operand) - no PE transposes
- softmax denominators ride a ones-column in the P@V stationary operand;
  1/x = exp(-ln x) on the Scalar engine, broadcast via a tiny PE matmul
- causal masking at 128-block granularity + a triangular-mask multiply on
  the diagonal blocks only
- tail out-projection split by fs-half so the fs0 matmuls overlap the
  final softmax-division chain
"""
import numpy as np

import concourse.bass as bass
import concourse.mybir as mybir
import concourse.tile as tile
from concourse.bass_utils import run_bass_kernel_spmd

P = 128
B, T, D = 2, 2048, 1024
H_LOCAL = 4          # heads per core
HD = 64              # head dim
F = H_LOCAL * HD     # 256 features per group
KO = D // P          # 8 contraction subtiles
NT = 512             # matmul moving width / PSUM bank
QJ = T // NT         # 4 q column tiles
KT = T // P          # 16 k row tiles
N_CORES = 8
LAG = 4              # S-matmul lookahead over P@V accumulation

f32 = mybir.dt.float32
f32r = mybir.dt.float32r
bf16 = mybir.dt.bfloat16

_uid = [0]


def _legalize_single_wait(nc):
    # This walrus build accepts only ONE sem wait per instruction; hoist
    # extra waits onto single-wait NoOps placed just before the instruction.
    for fn in nc.m.functions:
        for bb in fn.blocks:
            new_list = []
            changed = False
            for inst in bb.instructions:
                si = inst.sync_info
                if si is not None and len(si.on_wait) > 1:
                    waits = list(si.on_wait)
                    for w in waits[:-1]:
                        _uid[0] += 1
                        new_list.append(mybir.InstNoOp(
                            name=f"I-waitsplit-{_uid[0]}",
                            engine=inst.engine,
                            sync_info=mybir.SyncInfo(on_wait=[w], on_update=[]),
                        ))
                    inst.sync_info = mybir.SyncInfo(
                        on_wait=[waits[-1]], on_update=list(si.on_update))
                    changed = True
                new_list.append(inst)
            if changed:
                bb.instructions.clear()
                bb.instructions.extend(new_list)


def build_nc():
    nc = bass.Bass(trn_type="TRN2", target_bir_lowering=False, debug=False,
                   num_devices=N_CORES)
    # all inputs pre-cast to bf16 AND pre-arranged partition-major on the
    # host, so every DMA descriptor is a fat contiguous 4-8KB per-partition
    # run (512B descriptors were per-packet-overhead bound: 256KB took ~9us).
    xA = nc.dram_tensor("xA", [P, QJ, KO, NT], bf16, kind="ExternalInput").ap()
    wqA = nc.dram_tensor("wqA", [P, KO, F], bf16, kind="ExternalInput").ap()
    wkA = nc.dram_tensor("wkA", [P, KO, F], bf16, kind="ExternalInput").ap()
    wvA = nc.dram_tensor("wvA", [P, KO, F], bf16, kind="ExternalInput").ap()
    woA = nc.dram_tensor("woA", [P, F // P, D], bf16, kind="ExternalInput").ap()
    TRI = nc.dram_tensor("TRI", [P, P], bf16, kind="ExternalInput").ap()
    Z = nc.dram_tensor("Z", [T, D], bf16, kind="ExternalOutput").ap()

    w_r = {"q": wqA, "k": wkA, "v": wvA}

    with tile.TileContext(nc) as tc:
        with (
            tc.tile_pool(name="cw", bufs=1) as cw,
            tc.tile_pool(name="sb1", bufs=1) as sb1,
            tc.tile_pool(name="tp", bufs=4) as tp,
            tc.tile_pool(name="psS", bufs=2, space="PSUM") as psS,
            tc.tile_pool(name="psO", bufs=2, space="PSUM") as psO,
            tc.tile_pool(name="psM", bufs=2, space="PSUM") as psM,
        ):
            # ---- PE warmup: ~3.4us of dummy matmuls so the HAM clock-gate
            # un-throttles (1.2->2.4GHz) while the input DMAs are in flight ----
            warm = cw.tile([P, P], bf16, tag="warm", name="warm")
            nc.gpsimd.memset(warm[:], 0.0)
            for i in range(32):
                pw = psM.tile([P, P], f32, tag="m", name=f"warm{i}")
                nc.tensor.matmul(pw[:], warm[:], warm[:], start=True, stop=True)

            # ---- persistent constants / staging ----
            w_sb = {}
            for name in ("q", "k", "v"):
                w_sb[name] = sb1.tile([P, KO, F], bf16, tag=f"w{name}",
                                      name=f"w{name}")
            # qj-major so each per-qj DMA is one contiguous 8KB run per
            # partition (8KB descriptors instead of 1KB)
            xt = sb1.tile([P, QJ, KO, NT], bf16, tag="xt", name="xt")
            # DMA plan: one fat-descriptor transfer per tensor (128 x 4-8KB
            # descriptors each), weights on SP, x on ACT. Everything is
            # resident within ~10us.
            nc.sync.dma_start(w_sb["q"][:], w_r["q"])
            nc.scalar.dma_start(xt[:, 0], xA[:, 0])
            nc.sync.dma_start(w_sb["k"][:], w_r["k"])
            nc.sync.dma_start(w_sb["v"][:], w_r["v"])
            for qj in range(1, QJ):
                nc.scalar.dma_start(xt[:, qj], xA[:, qj])
            wo = cw.tile([P, F // P, D], bf16, tag="wo", name="wo")
            nc.sync.dma_start(wo[:], woA)
            tri = cw.tile([P, P], bf16, tag="tri", name="tri")
            nc.gpsimd.dma_start(tri[:], TRI)
            ones_r = cw.tile([1, HD], f32r, tag="ones", name="ones")
            nc.gpsimd.memset(ones_r[:].bitcast(f32), 1.0)

            # Q and K staged bf16, feature-major: subtile fs holds heads
            # (2fs, 2fs+1) at partition rows 0:64 / 64:128 — the natural
            # projection-PSUM layout. S-matmuls contract over K=64 rows via
            # tile_position so the two heads of a pair run CONCURRENTLY in
            # different PE row groups.
            qt = cw.tile([P, F // P, T], bf16, tag="qt", name="qt")
            kt2 = cw.tile([P, F // P, T], bf16, tag="kt2", name="kt2")

            # V with a ones column per head: [k-token, kt, head, 0:64]=V^T,
            # [..., 64]=1 (gives softmax denominators for free in P@V)
            vaug = cw.tile([P, KT, H_LOCAL, HD + 1], bf16, tag="vaug", name="vaug")
            nc.gpsimd.memset(vaug[:, :, :, HD:HD + 1], 1.0)

            ot = cw.tile([P, F // P, T], bf16, tag="ot", name="ot")

            def phase1_chunks(qj):
                # emission chunks (each ~8 PE matmuls) to splice between
                # attention pairs so the PE stream never drains
                sl = slice(qj * NT, (qj + 1) * NT)
                chunks = []

                def proj(name, fs):
                    # Q/K projection: [128 features, NT tokens] -> bf16 stage
                    def emit():
                        ps = psS.tile([P, NT], f32, tag="s",
                                      name=f"ps_{name}{fs}_{qj}")
                        for ko in range(KO):
                            nc.tensor.matmul(
                                ps[:], w_sb[name][:, ko, fs * P:(fs + 1) * P],
                                xt[:, qj, ko, :],
                                start=(ko == 0), stop=(ko == KO - 1))
                        dst = qt if name == "q" else kt2
                        nc.vector.tensor_copy(dst[:, fs, sl], ps[:])
                    return emit

                def vtchunk(kt):
                    # V^T directly: stationary = x chunk, moving = Wv
                    # -> [128 k-tokens, 256 features] accumulated over ko
                    def emit():
                        pv = psM.tile([P, F], f32, tag="m", name=f"pv{kt}")
                        loc = kt - 4 * qj
                        for ko in range(KO):
                            nc.tensor.matmul(
                                pv[:], xt[:, qj, ko, loc * P:(loc + 1) * P],
                                w_sb["v"][:, ko, :],
                                start=(ko == 0), stop=(ko == KO - 1))
                        nc.vector.tensor_copy(
                            vaug[:, kt, :, 0:HD],
                            pv.rearrange("p (h hd) -> p h hd", h=H_LOCAL))
                    return emit

                for name in ("q", "k"):
                    for fs in range(F // P):
                        chunks.append(proj(name, fs))
                for kt in range(4 * qj, 4 * qj + 4):
                    chunks.append(vtchunk(kt))
                return chunks

            def phase23(pr, qj, pending, fillers=None, late=None):
                # head pair (a, b) = (2pr, 2pr+1); S-matmuls for the two
                # heads are emitted back-to-back and run concurrently on the
                # PE array (K=64 each, row groups 0:64 / 64:128).
                a, b = 2 * pr, 2 * pr + 1
                n_ki = 4 * qj + 4
                po = {
                    a: psO.tile([HD + 1, NT], f32, tag="o", name=f"po{a}_{qj}"),
                    b: psO.tile([HD + 1, NT], f32, tag="o", name=f"po{b}_{qj}"),
                }
                pts = {}

                def s_step(ki):
                    # both heads' S-matmuls back-to-back: K=64 row groups
                    # 0:64 / 64:128 run concurrently on the PE array; one
                    # merged exp over the 2-bank PSUM tile halves the ACT
                    # instruction count.
                    col0 = 0 if ki < 4 * qj else (ki - 4 * qj) * P
                    N = NT - col0
                    ps2 = psS.tile([P, 2, NT], f32, tag="s",
                                   name=f"pss{pr}_{qj}_{ki}")
                    for j, h in ((0, a), (1, b)):
                        rows = slice(HD * j, HD * j + HD)
                        nc.tensor.matmul(
                            ps2[:, j, 0:N], kt2[rows, pr, ki * P:(ki + 1) * P],
                            qt[rows, pr, qj * NT + col0:(qj + 1) * NT],
                            start=True, stop=True)
                    pt2 = tp.tile([P, 2, NT], bf16, tag="pt", bufs=4,
                                  name=f"pt{pr}_{qj}_{ki}")
                    nc.scalar.activation(pt2[:, :, 0:N], ps2[:, :, 0:N],
                                         mybir.ActivationFunctionType.Exp,
                                         scale=0.125)
                    if ki >= 4 * qj:
                        nc.vector.tensor_mul(
                            pt2[:, :, 0:P], pt2[:, :, 0:P],
                            tri[:].rearrange("p (o n) -> p o n", o=1)
                            .broadcast_to([P, 2, P]))
                    pts[ki] = (pt2, col0, N)

                def o_step(ki):
                    pt2, col0, N = pts.pop(ki)
                    for j, h in ((0, a), (1, b)):
                        nc.tensor.matmul(
                            po[h][:, col0:NT], vaug[:, ki, h, :], pt2[:, j, 0:N],
                            start=(ki == 0), stop=(ki == n_ki - 1))

                for ki in range(n_ki + LAG):
                    if ki < n_ki:
                        s_step(ki)
                    if fillers and ki >= 1:
                        fillers.pop(0)()
                    if pending and ki == LAG:
                        # previous pair's divisions, emitted here so their PE
                        # broadcasts never head the PE stream while waiting
                        # on the reciprocal chain
                        pending.pop(0)()
                    if pending and ki == LAG + 2:
                        pending.pop(0)()
                    if late and ki >= n_ki:
                        late.pop(0)()
                    if ki >= LAG:
                        o_step(ki - LAG)
                while pending:
                    pending.pop(0)()

                def division(h):
                    def emit():
                        # divide by the ones-column sums; write head's OT
                        # slice. 1/x = exp(-ln x) on the Scalar engine (two
                        # LUT ops, ~2e-4 rel err) — no DVE reciprocal.
                        lg = tp.tile([1, NT], f32, tag="lg", bufs=2,
                                     name=f"lg{h}_{qj}")
                        nc.scalar.activation(lg[:], po[h][HD:HD + 1, :],
                                             mybir.ActivationFunctionType.Ln)
                        rrt = tp.tile([1, NT], f32r, tag="rr", bufs=2,
                                      name=f"rr{h}_{qj}")
                        nc.scalar.activation(rrt[:], lg[:],
                                             mybir.ActivationFunctionType.Exp,
                                             scale=-1.0)
                        so = tp.tile([HD, NT], f32, tag="so", bufs=2,
                                     name=f"so{h}_{qj}")
                        nc.vector.tensor_copy(so[:], po[h][0:HD, :])
                        pb = psM.tile([HD, NT], f32, tag="m", name=f"pb{h}_{qj}")
                        nc.tensor.matmul(pb[:], ones_r[:], rrt[:],
                                         start=True, stop=True)
                        nc.vector.tensor_mul(
                            ot[HD * (h % 2):HD * (h % 2) + HD, h // 2,
                               qj * NT:(qj + 1) * NT],
                            so[:], pb[:])
                    return emit
                return [division(a), division(b)]

            def phase4(qt_, tail=False):
                # partial out-projection in bf16. Mid-kernel: DVE casts, one
                # DMA per row-tile. Tail (after the last division): casts on
                # the now-idle ACT engine and per-dt half DMAs on alternating
                # queues so the dt0 half streams out while dt1 computes.
                zq = tp.tile([P, D], bf16, tag="z", bufs=2, name=f"zq{qt_}")
                pz2 = psS.tile([P, 2, NT], f32, tag="s",
                               name=f"pz2_{qt_}") if tail else None
                for dt in range(D // NT):
                    pz = pz2[:, dt] if tail else psM.tile(
                        [P, NT], f32, tag="m", name=f"pz{qt_}_{dt}")[:]
                    for fs in range(F // P):
                        nc.tensor.matmul(
                            pz, ot[:, fs, qt_ * P:(qt_ + 1) * P],
                            wo[:, fs, dt * NT:(dt + 1) * NT],
                            start=(fs == 0), stop=(fs == F // P - 1))
                    if tail:
                        if dt == 0:
                            nc.vector.tensor_copy(zq[:, 0:NT], pz)
                        else:
                            nc.scalar.copy(zq[:, NT:2 * NT], pz)
                        nc.sync.dma_start(
                            Z[qt_ * P:(qt_ + 1) * P, dt * NT:(dt + 1) * NT],
                            zq[:, dt * NT:(dt + 1) * NT])
                    else:
                        nc.vector.tensor_copy(zq[:, dt * NT:(dt + 1) * NT],
                                              pz)
                if not tail:
                    eng = nc.sync if qt_ % 2 == 0 else nc.scalar
                    eng.dma_start(Z[qt_ * P:(qt_ + 1) * P, :], zq[:])

            tailz = {}

            def p4a(qt_):
                def emit():
                    pz2 = psS.tile([P, 2, NT], f32, tag="s", name=f"tz{qt_}")
                    tailz[qt_] = pz2
                    for dt in range(D // NT):
                        nc.tensor.matmul(
                            pz2[:, dt], ot[:, 0, qt_ * P:(qt_ + 1) * P],
                            wo[:, 0, dt * NT:(dt + 1) * NT],
                            start=True, stop=False)
                return emit

            def p4b(qt_):
                pz2 = tailz[qt_]
                zq = tp.tile([P, D], bf16, tag="z", bufs=2, name=f"zqt{qt_}")
                for dt in range(D // NT):
                    nc.tensor.matmul(
                        pz2[:, dt], ot[:, 1, qt_ * P:(qt_ + 1) * P],
                        wo[:, 1, dt * NT:(dt + 1) * NT],
                        start=False, stop=True)
                    if dt == 0:
                        nc.vector.tensor_copy(zq[:, 0:NT], pz2[:, dt])
                    else:
                        nc.scalar.copy(zq[:, NT:2 * NT], pz2[:, dt])
                    nc.sync.dma_start(
                        Z[qt_ * P:(qt_ + 1) * P, dt * NT:(dt + 1) * NT],
                        zq[:, dt * NT:(dt + 1) * NT])

            pending = []
            chunks0 = phase1_chunks(0)
            for c in chunks0[:4]:       # q/k projections up front
                c()
            fillers0 = chunks0[4:]      # vT chunks ride inside pair0's loop
            p4_backlog = []
            for qj in range(QJ):
                splice = list(phase1_chunks(qj + 1)) if qj + 1 < QJ else []
                if qj == 2:
                    splice += p4_backlog[:4]       # phase4 of qj 0
                    p4_backlog = p4_backlog[4:]
                elif qj == 3:
                    splice += p4_backlog           # phase4 of qj 1 and 2
                    p4_backlog = []
                n_pr = H_LOCAL // 2
                for pr in range(n_pr):
                    fl = fillers0 if (qj == 0 and pr == 0) else None
                    lt = [p4a(12), p4a(13)] if (qj == 3 and pr == 1) else None
                    pending = phase23(pr, qj, pending, fillers=fl, late=lt)
                    k0 = (len(splice) * pr) // n_pr
                    k1 = (len(splice) * (pr + 1)) // n_pr
                    for c in splice[k0:k1]:
                        c()
                p4_backlog += [(lambda q=q: phase4(q))
                               for q in range(4 * qj, 4 * qj + 4)]
                if qj == 3:
                    p4_backlog = [lambda: p4b(12), lambda: p4b(13)] + [
                        (lambda q=q: phase4(q, tail=True))
                        for q in range(14, 16)]
            while pending:
                pending.pop(0)()
            for c in p4_backlog:
                c()

    _legalize_single_wait(nc)
    return nc


_TRI = None


def _make_in_maps(x, Wq, Wk, Wv, Wo):
    import ml_dtypes
    bf = ml_dtypes.bfloat16
    global _TRI
    if _TRI is None:
        # allowed[k_row, q_col] = q >= k  (upper-triangular incl. diagonal)
        _TRI = (np.arange(P)[None, :] >= np.arange(P)[:, None]).astype(bf)

    def warr(WT):
        # [D, F] -> [P, KO, F] partition-major (contiguous per partition)
        return np.ascontiguousarray(
            WT.reshape(KO, P, F).transpose(1, 0, 2)).astype(bf)

    in_maps = []
    for c in range(N_CORES):
        b, g = divmod(c, 4)
        sl = slice(g * F, (g + 1) * F)
        xT = np.asarray(x)[b].T  # [D, T]
        xa = np.ascontiguousarray(
            xT.reshape(KO, P, QJ, NT).transpose(1, 2, 0, 3)).astype(bf)
        WoT = np.asarray(Wo)[:, sl].T  # [F, D]
        woa = np.ascontiguousarray(
            WoT.reshape(F // P, P, D).transpose(1, 0, 2)).astype(bf)
        in_maps.append({
            "xA": xa,
            "wqA": warr(np.asarray(Wq)[sl, :].T),
            "wkA": warr(np.asarray(Wk)[sl, :].T),
            "wvA": warr(np.asarray(Wv)[sl, :].T),
            "woA": woa,
            "TRI": _TRI,
        })
    return in_maps


def run(x, Wq, Wk, Wv, Wo, trace=False, trace_cores=None):
    nc = build_nc()
    in_maps = _make_in_maps(x, Wq, Wk, Wv, Wo)
    res = run_bass_kernel_spmd(nc, in_maps, list(range(N_CORES)), trace=trace,
                               trace_cores=trace_cores)
    out = np.zeros((B, T, D), np.float32)
    for c in range(N_CORES):
        out[c // 4] += res.results[c]["Z"].astype(np.float32)
    return out, res


def kernel(x, Wq, Wk, Wv, Wo):
    try:
        out, _ = run(x, Wq, Wk, Wv, Wo)
    except Exception:
        # one retry for transient device errors (e.g. a wedged core from a
        # prior run)
        out, _ = run(x, Wq, Wk, Wv, Wo)
    return out
